# revision 24
# baseline (speedup 1.0000x reference)
"""Trainium2 Bass kernel for CrossAttentionBlock (nn_CrossAttentionBlock_12317966205103).

Sharding (v3b): 8 cores = 4 batches x 2 KV-halves. Each core computes
LN/K/V for its 2048 kv rows and attention of ALL 512 queries against
them, producing partial softmax numerators + denominators. A pair
ReduceScatter (bf16) sums the partials and hands each core its 256
output rows, which then run Wo/LN3/MLP locally (no other comms).

Device math (per core):
  z    = LN(point_features^T)            [D, 2048] bf16 (stats via ones-matmuls)
  kT   = Wk'^T z + c_k, then *= rk/8     [D, 2048] bf16
  v    = z^T Wv' stored 65-col head groups + ones col (denominator)
  qhT  = rms/weight-folded query proj    [D, 512] bf16
  per (head-pair, q-chunk, block): sT = kT_h^T qhT_h; e = exp(sT - 8);
    ctx_aug += [v_h | 1]^T e  (PSUM per block, SBUF f32 across blocks)
  ReduceScatter(pair) of [2, ctx] -> own 256-q ctx summed
  out_attn = (ctx/den)^T Wo + bo' + residual;  LN3;  gelu MLP;  sum.

bf16 matmul operands everywhere (FWL hides LDWEIGHTS); rsqrt via
exp(-0.5*ln(x)) so only the natural_log_exp table set is live; kT
pre-scaled by rk so exp ACTs batch [128,512]; v-bias folded host-side
into bo' = bo + c_v @ Wo.
"""

import os

import ml_dtypes
import numpy as np

import concourse.bass as bass
import concourse.tile as tile
from concourse import bacc, mybir
from concourse.bass_utils import run_bass_kernel_spmd
from concourse.masks import make_identity

F32 = mybir.dt.float32
BF16 = mybir.dt.bfloat16
NPBF = ml_dtypes.bfloat16
ALU = mybir.AluOpType
ACTF = mybir.ActivationFunctionType

D = 1024
N = 4096
NL = N // 2       # kv rows per core
QF = 512          # queries per core (attention)
KQ = 256          # output query rows per core
H = 16
HD = 64
FF = 4096         # mlp hidden
NB = 512          # n-block size
NBLK = NL // NB   # 4
S = NB // 128     # 4 n-subchunks per block
DC = D // 128     # 8 d-chunks
FC = 512          # mlp f-chunk
NEG_C = -8.0      # softmax stability shift (scores observed in [-8, 8])
LN8 = 2.0794415416798357
RG = [[0, 1], [2, 3], [4, 5], [6, 7]]

DEBUG = os.environ.get("BASSK_DEBUG", "0") == "1"

LN_EPS = 1e-5
RMS_EPS = 1e-6


def _emit(nc, tc, io, consts):
    # ---------- whole-program constants / survivors ----------
    ident = consts.tile([128, 128], BF16)
    make_identity(nc, ident[:])

    ones_f32 = consts.tile([128, 2], F32)
    nc.vector.memset(ones_f32[:], 1.0)
    ones_bf = consts.tile([128, 2], BF16)
    nc.vector.tensor_copy(ones_bf[:], ones_f32[:])

    negc = consts.tile([128, 1], F32)
    nc.vector.memset(negc[:], NEG_C)
    c_zero = consts.tile([128, 1], F32)
    nc.vector.memset(c_zero[:], 0.0)
    c_ln_eps = consts.tile([128, 1], F32)
    nc.vector.memset(c_ln_eps[:], LN_EPS)
    c_rms_eps = consts.tile([128, 1], F32)
    nc.vector.memset(c_rms_eps[:], RMS_EPS)
    c_mln8 = consts.tile([128, 1], F32)
    nc.vector.memset(c_mln8[:], -LN8)

    ck_sb = consts.tile([128, DC], F32)
    nc.sync.dma_start(ck_sb[:], io["ck"])
    cq_sb = consts.tile([128, DC], F32)
    nc.sync.dma_start(cq_sb[:], io["cq"])
    wqk_sb = consts.tile([128, DC], F32)
    nc.sync.dma_start(wqk_sb[:], io["wqk"])
    c1_sb = consts.tile([128, FF // 128], F32)
    nc.sync.dma_start(c1_sb[:], io["c1"])

    def bcast_row(dst, src_ap):
        nc.gpsimd.dma_start(
            out=dst,
            in_=bass.AP(tensor=src_ap.tensor, offset=src_ap.offset,
                        ap=[[0, 128], src_ap.ap[1]]),
        )

    qhT = consts.tile([128, DC, QF], BF16)        # \hat q ^T (512 q)
    # partial ctx accumulators f32: rows 0-63 ctx, row 64 denominator
    ctxA = consts.tile([128, DC, QF], F32)        # even heads
    ctxB = consts.tile([128, DC, QF], F32)        # odd heads
    out_attn = consts.tile([128, 2, D], F32)
    z3T = consts.tile([128, DC, KQ], BF16)

    # DRAM bounce for the pair ReduceScatter of ctx partials (bf16)
    dram_cm = tc.tile_pool(name="dram", bufs=1, space="DRAM")
    dram = dram_cm.__enter__()
    cc_in = dram.tile([2, 2, 128, DC, KQ], BF16)   # [q-half, A/B, ...]
    cc_out = dram.tile([1, 2, 128, DC, KQ], BF16)

    # ================= attention super-phase =================
    with (
        tc.tile_pool(name="wpool", bufs=2) as wpool,
        tc.tile_pool(name="zpool", bufs=2) as zpool,
        tc.tile_pool(name="ktp", bufs=2) as ktp,
        tc.tile_pool(name="vp", bufs=2) as vp,
        tc.tile_pool(name="scratch", bufs=2) as scr,
        tc.tile_pool(name="expp", bufs=3) as expp,
        tc.tile_pool(name="rowk", bufs=2) as rowk,
        tc.tile_pool(name="rkp", bufs=2) as rkp,
        tc.tile_pool(name="psP", bufs=2, space="PSUM") as psP,
        tc.tile_pool(name="psS", bufs=2, space="PSUM") as psS,
        tc.tile_pool(name="psC", bufs=2, space="PSUM") as psC,
        tc.tile_pool(name="psR", bufs=2, space="PSUM") as psR,
    ):
        # ---------- helpers ----------
        def ln_stats(x_sb, ncols, t, pool):
            ps_s = psR.tile([1, ncols], F32, tag="psR", name="ps_s" + t)
            ps_q = psR.tile([1, ncols], F32, tag="psR", name="ps_q" + t)
            for cc in range(DC):
                sq = scr.tile([128, ncols], BF16, tag="sq" + t)
                nc.vector.tensor_tensor(sq[:], x_sb[:, cc, :], x_sb[:, cc, :],
                                        ALU.mult)
                nc.tensor.matmul(ps_s[:], ones_bf[:, 0:1], x_sb[:, cc, :],
                                 start=(cc == 0), stop=(cc == DC - 1))
                nc.tensor.matmul(ps_q[:], ones_bf[:, 0:1], sq[:],
                                 start=(cc == 0), stop=(cc == DC - 1))
            st = pool.tile([1, 2, ncols], F32, tag="st" + t)
            mu, msq = st[:, 0, :], st[:, 1, :]
            nc.vector.tensor_scalar_mul(mu, ps_s[:], 1.0 / D)
            nc.vector.tensor_scalar_mul(msq, ps_q[:], 1.0 / D)
            bfr = pool.tile([1, 2, ncols], BF16, tag="bfr" + t)
            rln, mrow = bfr[:, 0, :], bfr[:, 1, :]
            # var = msq - mu^2 (mu^2 via the bf16 rln slot; |mu|<<1 so fine)
            nc.vector.tensor_tensor(rln, mu, mu, ALU.mult)
            nc.vector.tensor_tensor(msq, msq, rln, ALU.subtract)
            nc.scalar.activation(out=msq, in_=msq, func=ACTF.Ln,
                                 bias=c_ln_eps[0:1, 0:1], scale=1.0)
            nc.scalar.activation(out=rln, in_=msq, func=ACTF.Exp,
                                 bias=c_zero[0:1, 0:1], scale=-0.5)
            nc.vector.tensor_tensor(mrow, mu, rln, ALU.mult)
            nc.vector.tensor_scalar_mul(mrow, mrow, -1.0)
            return rln, mrow

        def normalize(x_sb, z_sb, rln, mrow, ncols, t):
            rb = scr.tile([128, 2, ncols], BF16, tag="rb" + t)
            nc.gpsimd.partition_broadcast(rb[:, 0, :], rln)
            nc.gpsimd.partition_broadcast(rb[:, 1, :], mrow)
            nc.vector.tensor_tensor(
                z_sb[:], x_sb[:],
                rb[:, 0, :].unsqueeze(1).to_broadcast([128, DC, ncols]),
                ALU.mult)
            nc.vector.tensor_tensor(
                z_sb[:], z_sb[:],
                rb[:, 1, :].unsqueeze(1).to_broadcast([128, DC, ncols]),
                ALU.add)

        # prefetch all pf blocks early; the sync ring then frees up for the
        # MLP weight stream which follows it in sync-engine program order
        pf_pre = []
        for j in range(min(2, NBLK)):
            pfj = zpool.tile([128, DC, NB], BF16, tag="pf")
            nc.sync.dma_start(pfj[:], io["pf"][j])
            pf_pre.append(pfj)

        # ---------- phase Q: qhT (512 queries) ----------
        with (
            tc.tile_pool(name="qph", bufs=1) as qph,
            tc.tile_pool(name="rowq", bufs=1) as rowq,
        ):
            wq_sb = wpool.tile([128, DC, D], BF16, tag="w")
            nc.sync.dma_start(wq_sb[:], io["wq"])
            qt_sb = qph.tile([128, DC, QF], BF16, tag="qt", name="qt")
            nc.sync.dma_start(qt_sb[:], io["qt"])

            rln_q, mrow_q = ln_stats(qt_sb, QF, "q", rowq)
            normalize(qt_sb, qt_sb, rln_q, mrow_q, QF, "q")

            qraw = qph.tile([128, DC, QF], BF16, tag="qraw")
            for dc in range(DC):
                pq = psP.tile([128, QF], F32, tag="psP")
                for cc in range(DC):
                    nc.tensor.matmul(pq[:],
                                     wq_sb[:, cc, dc * 128:(dc + 1) * 128],
                                     qt_sb[:, cc, :], start=(cc == 0),
                                     stop=(cc == DC - 1))
                nc.vector.tensor_scalar_add(qraw[:, dc, :], pq[:],
                                            cq_sb[:, dc:dc + 1])
            psq = psR.tile([1, QF], F32, tag="psR")
            for dc in range(DC):
                sqq = scr.tile([128, QF], BF16, tag="sqq")
                nc.vector.tensor_tensor(sqq[:], qraw[:, dc, :], qraw[:, dc, :],
                                        ALU.mult)
                nc.tensor.matmul(psq[:], ones_bf[:, 0:1], sqq[:],
                                 start=(dc == 0), stop=(dc == DC - 1))
            rq = rowq.tile([1, QF], F32, tag="rq")
            nc.scalar.activation(out=rq[:], in_=psq[:], func=ACTF.Ln,
                                 bias=c_rms_eps[0:1, 0:1], scale=1.0 / D)
            nc.scalar.activation(out=rq[:], in_=rq[:], func=ACTF.Exp,
                                 bias=c_zero[0:1, 0:1], scale=-0.5)
            rq_bc = qph.tile([128, QF], F32, tag="rqbc")
            nc.gpsimd.partition_broadcast(rq_bc[:], rq[:])
            for dc in range(DC):
                nc.vector.scalar_tensor_tensor(
                    out=qhT[:, dc, :], in0=qraw[:, dc, :],
                    scalar=wqk_sb[:, dc:dc + 1], in1=rq_bc[:],
                    op0=ALU.mult, op1=ALU.mult)

        # ---------- main: block-pipelined projections + attention ----------
        wk_sb = wpool.tile([128, DC, D], BF16, tag="w")
        nc.sync.dma_start(wk_sb[:], io["wk"])
        wv_sb = wpool.tile([128, DC, D], BF16, tag="w")
        nc.sync.dma_start(wv_sb[:], io["wv"])

        def proj_block(j):
            if j < len(pf_pre):
                pf = pf_pre[j]
            else:
                pf = zpool.tile([128, DC, NB], BF16, tag="pf")
                nc.sync.dma_start(pf[:], io["pf"][j])
            rln, mrow = ln_stats(pf, NB, "kv", rowk)
            normalize(pf, pf, rln, mrow, NB, "kv")
            z = pf
            kT = ktp.tile([128, DC, NB], BF16, tag="kt")
            for dc in range(DC):
                pk = psP.tile([128, NB], F32, tag="psP")
                for cc in range(DC):
                    nc.tensor.matmul(
                        pk[:], wk_sb[:, cc, dc * 128:(dc + 1) * 128],
                        z[:, cc, :], start=(cc == 0), stop=(cc == DC - 1))
                nc.vector.tensor_scalar_add(kT[:, dc, :], pk[:],
                                            ck_sb[:, dc:dc + 1])
            v_j = vp.tile([128, S, H * 65], BF16, tag="v")
            ones_dst = bass.AP(
                tensor=v_j[:].tensor, offset=v_j[:, 0, 64:65].offset,
                ap=[v_j[:].ap[0], [H * 65, S], [65, H]])
            nc.vector.tensor_copy(
                ones_dst,
                ones_bf[:, 0:1].unsqueeze(1).to_broadcast([128, S, H]))
            for s4 in range(S):
                for dh in range(2):
                    pv = psP.tile([128, 512], F32, tag="psP")
                    for cc in range(DC):
                        nc.tensor.matmul(
                            pv[:], z[:, cc, s4 * 128:(s4 + 1) * 128],
                            wv_sb[:, cc, dh * 512:(dh + 1) * 512],
                            start=(cc == 0), stop=(cc == DC - 1))
                    dst = bass.AP(
                        tensor=v_j[:].tensor,
                        offset=v_j[:, s4, dh * 8 * 65:dh * 8 * 65 + 1].offset,
                        ap=[v_j[:].ap[0], [65, 8], [1, 64]])
                    nc.vector.tensor_copy(dst, pv[:])
            # rk = (1/8)*rsqrt(mean(k^2)+eps), then pre-scale kT by it
            rk_row = rkp.tile([1, NB], F32, tag="rk")
            prk = psR.tile([1, NB], F32, tag="psR")
            for dc in range(DC):
                sqk = scr.tile([128, NB], BF16, tag="sqk")
                nc.vector.tensor_tensor(sqk[:], kT[:, dc, :], kT[:, dc, :],
                                        ALU.mult)
                nc.tensor.matmul(prk[:], ones_bf[:, 0:1], sqk[:],
                                 start=(dc == 0), stop=(dc == DC - 1))
            nc.scalar.activation(out=rk_row[:], in_=prk[:], func=ACTF.Ln,
                                 bias=c_rms_eps[0:1, 0:1], scale=1.0 / D)
            nc.scalar.activation(out=rk_row[:], in_=rk_row[:], func=ACTF.Exp,
                                 bias=c_mln8[0:1, 0:1], scale=-0.5)
            rk_bc = rkp.tile([128, NB], F32, tag="rkbc")
            nc.gpsimd.partition_broadcast(rk_bc[:], rk_row[:])
            for dc in range(DC):
                nc.vector.tensor_tensor(kT[:, dc, :], kT[:, dc, :], rk_bc[:],
                                        ALU.mult)
            return kT, v_j

        def attn_block(j, kT, v_j):
            for hp in range(DC):
                for qc in range(2):
                    ql = qc * KQ
                    pc = psC.tile([128, 2, KQ], F32, tag="psC")
                    for sp in range(S // 2):
                        pa2 = psS.tile([128, 2, KQ], F32, tag="psS")
                        pb2 = psS.tile([128, 2, KQ], F32, tag="psS")
                        for t in range(2):
                            nlo = (sp * 2 + t) * 128
                            nc.tensor.matmul(
                                pa2[:, t, :], kT[0:64, hp, nlo:nlo + 128],
                                qhT[0:64, hp, ql:ql + KQ], start=True,
                                stop=True, tile_position=(0, 0))
                            nc.tensor.matmul(
                                pb2[:, t, :], kT[64:128, hp, nlo:nlo + 128],
                                qhT[64:128, hp, ql:ql + KQ], start=True,
                                stop=True, tile_position=(64, 0))
                        es = expp.tile([128, 2, 2, KQ], BF16, tag="es")
                        nc.scalar.activation(out=es[:, 0], in_=pa2[:],
                                             func=ACTF.Exp, bias=negc[:],
                                             scale=1.0)
                        nc.scalar.activation(out=es[:, 1], in_=pb2[:],
                                             func=ACTF.Exp, bias=negc[:],
                                             scale=1.0)
                        for t in range(2):
                            ns = sp * 2 + t
                            for hh in range(2):
                                h = 2 * hp + hh
                                nc.tensor.matmul(
                                    pc[0:65, hh, :],
                                    v_j[:, ns, h * 65:(h + 1) * 65],
                                    es[:, hh, t, :],
                                    start=(sp == 0 and t == 0),
                                    stop=(sp == S // 2 - 1 and t == 1))
                    for hh, ctx_acc in ((0, ctxA), (1, ctxB)):
                        if j == 0:
                            nc.vector.tensor_copy(
                                ctx_acc[0:65, hp, ql:ql + KQ],
                                pc[0:65, hh, :])
                        else:
                            nc.vector.tensor_tensor(
                                ctx_acc[0:65, hp, ql:ql + KQ],
                                ctx_acc[0:65, hp, ql:ql + KQ],
                                pc[0:65, hh, :], ALU.add)

        pending = None
        for j in range(NBLK):
            cur = proj_block(j)
            if pending is not None:
                attn_block(*pending)
            pending = (j, *cur)
        attn_block(*pending)

        # ---------- pair ReduceScatter of ctx partials ----------
        for qc in range(2):
            ql = qc * KQ
            nc.gpsimd.dma_start(cc_in[qc, 0], ctxA[:, :, ql:ql + KQ])
            nc.gpsimd.dma_start(cc_in[qc, 1], ctxB[:, :, ql:ql + KQ])
        nc.gpsimd.collective_compute(
            "ReduceScatter", ALU.add, replica_groups=RG,
            ins=[cc_in.opt()], outs=[cc_out.opt()],
        )

    # ---------- normalize ctx, Wo projection, residual, LN3 ----------
    with (
        tc.tile_pool(name="late", bufs=1) as latep,
        tc.tile_pool(name="lscr", bufs=2) as lscr,
        tc.tile_pool(name="psW", bufs=2, space="PSUM") as psW,
        tc.tile_pool(name="psT", bufs=2, space="PSUM") as psT,
    ):
        bo_bc = latep.tile([128, D], F32)
        bcast_row(bo_bc[:], io["bo_row"])
        qres_sb = latep.tile([128, 2, D], F32)
        nc.scalar.dma_start(qres_sb[:], io["qres"])
        wo_sb = latep.tile([128, DC, D], BF16)
        nc.scalar.dma_start(wo_sb[:], io["wo"])

        ctxAs = latep.tile([128, DC, KQ], F32)
        ctxBs = latep.tile([128, DC, KQ], F32)
        nc.gpsimd.dma_start(ctxAs[:], cc_out[0, 0])
        nc.gpsimd.dma_start(ctxBs[:], cc_out[0, 1])

        cxh = latep.tile([128, DC, KQ], BF16)
        dinv = latep.tile([1, 2, DC, KQ], F32)
        nc.vector.reciprocal(dinv[:, 0], ctxAs[64:65, :, :])
        nc.vector.reciprocal(dinv[:, 1], ctxBs[64:65, :, :])
        for h in range(H):
            hp, hh = h // 2, h % 2
            ctx_acc = ctxAs if hh == 0 else ctxBs
            rb = lscr.tile([128, KQ], F32, tag="recb")
            nc.gpsimd.partition_broadcast(rb[:], dinv[:, hh, hp, :])
            lo = hh * 64
            nc.vector.tensor_tensor(cxh[lo:lo + 64, hp, :],
                                    ctx_acc[0:64, hp, :],
                                    rb[0:64, :], ALU.mult)

        for s in range(2):
            for dh in range(2):
                po = psW.tile([128, 512], F32, tag="psW")
                for dc in range(DC):
                    nc.tensor.matmul(po[:], cxh[:, dc, s * 128:(s + 1) * 128],
                                     wo_sb[:, dc, dh * 512:(dh + 1) * 512],
                                     start=(dc == 0), stop=(dc == DC - 1))
                nc.vector.tensor_tensor(
                    out_attn[:, s, dh * 512:(dh + 1) * 512], po[:],
                    bo_bc[:, dh * 512:(dh + 1) * 512], ALU.add)
            nc.vector.tensor_tensor(out_attn[:, s, :], out_attn[:, s, :],
                                    qres_sb[:, s, :], ALU.add)

        # LN3 + transpose to z3T
        for s in range(2):
            stats = lscr.tile([128, 2, 6], F32, tag="bn3")
            nc.vector.bn_stats(stats[:, 0, :], out_attn[:, s, 0:512])
            nc.vector.bn_stats(stats[:, 1, :], out_attn[:, s, 512:1024])
            mv = lscr.tile([128, 2], F32, tag="mv3")
            nc.vector.bn_aggr(mv[:], stats[:])
            rstd = lscr.tile([128, 1], F32, tag="rstd3")
            nc.scalar.activation(out=rstd[:], in_=mv[:, 1:2], func=ACTF.Ln,
                                 bias=c_ln_eps[:], scale=1.0)
            nc.scalar.activation(out=rstd[:], in_=rstd[:], func=ACTF.Exp,
                                 bias=c_zero[:], scale=-0.5)
            nbias = lscr.tile([128, 1], F32, tag="nb3")
            nc.vector.tensor_tensor(nbias[:], mv[:, 0:1], rstd[:], ALU.mult)
            nc.vector.tensor_scalar_mul(nbias[:], nbias[:], -1.0)
            z3 = lscr.tile([128, 2, 512], BF16, tag="z3")
            for half in range(2):
                nc.scalar.activation(
                    out=z3[:, half, :],
                    in_=out_attn[:, s, half * 512:(half + 1) * 512],
                    func=ACTF.Identity, bias=nbias[:], scale=rstd[:])
            for dc in range(DC):
                pt = psT.tile([128, 128], BF16, tag="psT")
                nc.tensor.transpose(
                    pt[:], z3[:, dc // 4, (dc % 4) * 128:(dc % 4 + 1) * 128],
                    ident[:])
                nc.vector.tensor_copy(z3T[:, dc, s * 128:(s + 1) * 128], pt[:])

        if DEBUG:
            nc.sync.dma_start(io["dbg_ctx"], cxh[:])
            nc.sync.dma_start(io["dbg_attn"], out_attn[:])

    # ================= MLP super-phase =================
    with (
        tc.tile_pool(name="mw", bufs=2) as mw,
        tc.tile_pool(name="gt", bufs=2) as gtp,
        tc.tile_pool(name="mrow2", bufs=1) as mrow2,
        tc.tile_pool(name="psH", bufs=2, space="PSUM") as psH,
        tc.tile_pool(name="psO", bufs=1, space="PSUM") as psO,
    ):
        b2_bc = mrow2.tile([128, D], F32)
        nc.gpsimd.dma_start(
            out=b2_bc[:],
            in_=bass.AP(tensor=io["b2_row"].tensor, offset=io["b2_row"].offset,
                        ap=[[0, 128], io["b2_row"].ap[1]]),
        )
        pouts = {}
        for s in range(2):
            for dh in range(2):
                pouts[(s, dh)] = psO.tile([128, 512], F32, tag=f"po{s}{dh}",
                                          name=f"po{s}{dh}")
        nfc = FF // FC  # 8
        for fc in range(nfc):
            w1c = mw.tile([128, DC, FC], BF16, tag="w1")
            nc.sync.dma_start(w1c[:], io["w1"][fc])
            w2c = mw.tile([128, FC // 128, D], BF16, tag="w2")
            nc.sync.dma_start(w2c[:], io["w2"][fc])
            gt = gtp.tile([128, FC // 128, KQ], BF16, tag="gt")
            for fp in range(2):
                ph = psH.tile([128, 2, KQ], F32, tag="psH")
                for fi in range(2):
                    fs = fp * 2 + fi
                    for cc in range(DC):
                        nc.tensor.matmul(
                            ph[:, fi, :], w1c[:, cc, fs * 128:(fs + 1) * 128],
                            z3T[:, cc, :], start=(cc == 0), stop=(cc == DC - 1))
                fidx0 = fc * (FC // 128) + fp * 2
                for fi in range(2):
                    nc.scalar.activation(
                        out=gt[:, fp * 2 + fi, :], in_=ph[:, fi, :],
                        func=ACTF.Gelu,
                        bias=c1_sb[:, fidx0 + fi:fidx0 + fi + 1], scale=1.0)
            for s in range(2):
                for dh in range(2):
                    for fs in range(FC // 128):
                        nc.tensor.matmul(
                            pouts[(s, dh)][:], gt[:, fs, s * 128:(s + 1) * 128],
                            w2c[:, fs, dh * 512:(dh + 1) * 512],
                            start=(fc == 0 and fs == 0),
                            stop=(fc == nfc - 1 and fs == FC // 128 - 1))

        for s in range(2):
            for dh in range(2):
                sl = slice(dh * 512, (dh + 1) * 512)
                nc.vector.tensor_tensor(out_attn[:, s, sl], pouts[(s, dh)][:],
                                        out_attn[:, s, sl], ALU.add)
            nc.vector.tensor_tensor(out_attn[:, s, :], out_attn[:, s, :],
                                    b2_bc[:], ALU.add)
        nc.sync.dma_start(io["out"], out_attn[:])

    dram_cm.__exit__(None, None, None)


def build():
    nc = bacc.Bacc("TRN2", target_bir_lowering=False, debug=False,
                   num_devices=8)
    io = {}
    io["pf"] = [
        nc.dram_tensor(f"pf{j}", [128, DC, NB], BF16, kind="ExternalInput").ap()
        for j in range(NBLK)
    ]
    io["qt"] = nc.dram_tensor("qt", [128, DC, QF], BF16, kind="ExternalInput").ap()
    io["qres"] = nc.dram_tensor("qres", [128, 2, D], F32, kind="ExternalInput").ap()
    for w in ["wq", "wk", "wv", "wo"]:
        io[w] = nc.dram_tensor(w, [128, DC, D], BF16, kind="ExternalInput").ap()
    io["w1"] = [
        nc.dram_tensor(f"w1_{i}", [128, DC, FC], BF16, kind="ExternalInput").ap()
        for i in range(FF // FC)
    ]
    io["w2"] = [
        nc.dram_tensor(f"w2_{i}", [128, FC // 128, D], BF16,
                       kind="ExternalInput").ap()
        for i in range(FF // FC)
    ]
    io["ck"] = nc.dram_tensor("ck", [128, DC], F32, kind="ExternalInput").ap()
    io["cq"] = nc.dram_tensor("cq", [128, DC], F32, kind="ExternalInput").ap()
    io["wqk"] = nc.dram_tensor("wqk", [128, DC], F32, kind="ExternalInput").ap()
    io["c1"] = nc.dram_tensor("c1", [128, FF // 128], F32, kind="ExternalInput").ap()
    io["bo_row"] = nc.dram_tensor("bo_row", [1, D], F32, kind="ExternalInput").ap()
    io["b2_row"] = nc.dram_tensor("b2_row", [1, D], F32, kind="ExternalInput").ap()
    io["out"] = nc.dram_tensor("out", [128, 2, D], F32, kind="ExternalOutput").ap()
    if DEBUG:
        for name, shape, dt in [
            ("dbg_ctx", [128, DC, KQ], BF16),
            ("dbg_attn", [128, 2, D], F32),
        ]:
            io[name] = nc.dram_tensor(name, shape, dt, kind="ExternalOutput").ap()

    with tile.TileContext(nc) as tc:
        with tc.tile_pool(name="consts", bufs=1) as consts:
            _emit(nc, tc, io, consts)
    nc.compile()
    return nc


def prep_core_inputs(inputs, core):
    """Host-side fold + shard + relayout for one core."""
    b, half = core // 2, core % 2
    f32 = np.float32
    qt_full = np.asarray(inputs["query_tokens"], f32)
    pf_full = np.asarray(inputs["point_features"], f32)
    Wq = np.asarray(inputs["Wq"], f32)
    Wk = np.asarray(inputs["Wk"], f32)
    Wv = np.asarray(inputs["Wv"], f32)
    Wo = np.asarray(inputs["Wo"], f32)
    W1 = np.asarray(inputs["W1"], f32)
    W2 = np.asarray(inputs["W2"], f32)
    g_q, b_q = np.asarray(inputs["ln_q_g"], f32), np.asarray(inputs["ln_q_b"], f32)
    g_kv, b_kv = np.asarray(inputs["ln_kv_g"], f32), np.asarray(inputs["ln_kv_b"], f32)
    g_m, b_m = np.asarray(inputs["ln_mlp_g"], f32), np.asarray(inputs["ln_mlp_b"], f32)

    Wqp = g_q[:, None] * Wq
    c_q = b_q @ Wq + np.asarray(inputs["bq"], f32)
    Wkp = g_kv[:, None] * Wk
    c_k = b_kv @ Wk + np.asarray(inputs["bk"], f32)
    Wvp = g_kv[:, None] * Wv
    c_v = b_kv @ Wv + np.asarray(inputs["bv"], f32)
    W1p = g_m[:, None] * W1
    c_1 = b_m @ W1 + np.asarray(inputs["b1"], f32)
    wqk = (np.asarray(inputs["rms_q_w"], f32) * np.asarray(inputs["rms_k_w"], f32))
    bo_f = np.asarray(inputs["bo"], f32) + c_v @ Wo   # fold v-bias into bo

    q_res = qt_full[b, half * KQ:(half + 1) * KQ]          # own 256 rows
    pfT = np.ascontiguousarray(pf_full[b].T)               # [D, N]
    qT = np.ascontiguousarray(qt_full[b].T)                # [D, 512] all queries

    def part_major(w, dt=NPBF):  # [D, X] -> [128, D//128, X]
        return np.ascontiguousarray(
            w.reshape(DC, 128, -1).transpose(1, 0, 2).astype(dt))

    m = {}
    # this core's kv half: blocks [half*NBLK, half*NBLK+NBLK)
    pf_dev = pfT.reshape(DC, 128, N // NB, NB).transpose(2, 1, 0, 3)
    for j in range(NBLK):
        m[f"pf{j}"] = np.ascontiguousarray(pf_dev[half * NBLK + j].astype(NPBF))
    m["qt"] = part_major(qT)
    m["qres"] = np.ascontiguousarray(q_res.reshape(2, 128, D).transpose(1, 0, 2))
    m["wq"] = part_major(Wqp)
    m["wk"] = part_major(Wkp)
    m["wv"] = part_major(Wvp)
    m["wo"] = part_major(Wo)
    w1_dev = part_major(W1p)                               # [128, 8, 4096]
    for i in range(FF // FC):
        m[f"w1_{i}"] = np.ascontiguousarray(w1_dev[:, :, i * FC:(i + 1) * FC])
    w2_dev = np.ascontiguousarray(
        W2.reshape(FF // 128, 128, D).transpose(1, 0, 2).astype(NPBF))
    for i in range(FF // FC):
        m[f"w2_{i}"] = np.ascontiguousarray(
            w2_dev[:, i * (FC // 128):(i + 1) * (FC // 128), :])
    m["ck"] = np.ascontiguousarray(c_k.reshape(DC, 128).T)
    m["cq"] = np.ascontiguousarray(c_q.reshape(DC, 128).T)
    m["wqk"] = np.ascontiguousarray(wqk.reshape(DC, 128).T)
    m["c1"] = np.ascontiguousarray(c_1.reshape(FF // 128, 128).T)
    m["bo_row"] = bo_f.reshape(1, D)
    m["b2_row"] = np.asarray(inputs["b2"], f32).reshape(1, D)
    return m


_NC_CACHE = None


def run_cores(inputs, **kw):
    global _NC_CACHE
    if _NC_CACHE is None:
        _NC_CACHE = build()
    in_maps = [prep_core_inputs(inputs, c) for c in range(8)]
    return run_bass_kernel_spmd(_NC_CACHE, in_maps, core_ids=list(range(8)), **kw)


def kernel(**inputs):
    res = run_cores(inputs)
    B, K = 4, 512
    out = np.zeros((B, K, D), np.float32)
    for c in range(8):
        b, half = c // 2, c % 2
        o = res.results[c]["out"]                          # [128, 2, 1024]
        out[b, half * KQ:(half + 1) * KQ] = o.transpose(1, 0, 2).reshape(KQ, D)
    return out


# revision 26
# speedup vs baseline: 1.1451x; 1.1451x over previous
"""Trainium2 Bass kernel for CrossAttentionBlock (nn_CrossAttentionBlock_12317966205103).

Sharding (v3b): 8 cores = 4 batches x 2 KV-halves. Each core computes
LN/K/V for its 2048 kv rows and attention of ALL 512 queries against
them, producing partial softmax numerators + denominators. A pair
ReduceScatter (bf16) sums the partials and hands each core its 256
output rows, which then run Wo/LN3/MLP locally (no other comms).

Device math (per core):
  z    = LN(point_features^T)            [D, 2048] bf16 (stats via ones-matmuls)
  kT   = Wk'^T z + c_k, then *= rk/8     [D, 2048] bf16
  v    = z^T Wv' stored 65-col head groups + ones col (denominator)
  qhT  = rms/weight-folded query proj    [D, 512] bf16
  per (head-pair, q-chunk, block): sT = kT_h^T qhT_h; e = exp(sT - 8);
    ctx_aug += [v_h | 1]^T e  (PSUM per block, SBUF f32 across blocks)
  ReduceScatter(pair) of [2, ctx] -> own 256-q ctx summed
  out_attn = (ctx/den)^T Wo + bo' + residual;  LN3;  gelu MLP;  sum.

bf16 matmul operands everywhere (FWL hides LDWEIGHTS); rsqrt via
exp(-0.5*ln(x)) so only the natural_log_exp table set is live; kT
pre-scaled by rk so exp ACTs batch [128,512]; v-bias folded host-side
into bo' = bo + c_v @ Wo.
"""

import os

import ml_dtypes
import numpy as np

import concourse.bass as bass
import concourse.tile as tile
from concourse import bacc, mybir
from concourse.bass_utils import run_bass_kernel_spmd
from concourse.masks import make_identity

F32 = mybir.dt.float32
BF16 = mybir.dt.bfloat16
NPBF = ml_dtypes.bfloat16
ALU = mybir.AluOpType
ACTF = mybir.ActivationFunctionType

D = 1024
N = 4096
NL = N // 2       # kv rows per core
QF = 512          # queries per core (attention)
KQ = 256          # output query rows per core
H = 16
HD = 64
FF = 4096         # mlp hidden
NB = 512          # n-block size
NBLK = NL // NB   # 4
S = NB // 128     # 4 n-subchunks per block
DC = D // 128     # 8 d-chunks
FC = 512          # mlp f-chunk
NEG_C = -8.0      # softmax stability shift (scores observed in [-8, 8])
LN8 = 2.0794415416798357
RG = [[0, 1], [2, 3], [4, 5], [6, 7]]

DEBUG = os.environ.get("BASSK_DEBUG", "0") == "1"

LN_EPS = 1e-5
RMS_EPS = 1e-6


def _emit(nc, tc, io, consts):
    # ---------- whole-program constants / survivors ----------
    ident = consts.tile([128, 128], BF16)
    make_identity(nc, ident[:])

    ones_f32 = consts.tile([128, 2], F32)
    nc.vector.memset(ones_f32[:], 1.0)
    ones_bf = consts.tile([128, 2], BF16)
    nc.vector.tensor_copy(ones_bf[:], ones_f32[:])

    negc = consts.tile([128, 1], F32)
    nc.vector.memset(negc[:], NEG_C)
    c_zero = consts.tile([128, 1], F32)
    nc.vector.memset(c_zero[:], 0.0)
    c_ln_eps = consts.tile([128, 1], F32)
    nc.vector.memset(c_ln_eps[:], LN_EPS)
    c_rms_eps = consts.tile([128, 1], F32)
    nc.vector.memset(c_rms_eps[:], RMS_EPS)
    c_mln8 = consts.tile([128, 1], F32)
    nc.vector.memset(c_mln8[:], -LN8)

    ck_sb = consts.tile([128, DC], F32)
    nc.sync.dma_start(ck_sb[:], io["ck"])
    cq_sb = consts.tile([128, DC], F32)
    nc.sync.dma_start(cq_sb[:], io["cq"])
    wqk_sb = consts.tile([128, DC], F32)
    nc.sync.dma_start(wqk_sb[:], io["wqk"])
    c1_sb = consts.tile([128, FF // 128], F32)
    nc.sync.dma_start(c1_sb[:], io["c1"])

    def bcast_row(dst, src_ap):
        nc.gpsimd.dma_start(
            out=dst,
            in_=bass.AP(tensor=src_ap.tensor, offset=src_ap.offset,
                        ap=[[0, 128], src_ap.ap[1]]),
        )

    qhT = consts.tile([128, DC, QF], BF16)        # \hat q ^T (512 q)
    # partial ctx accumulators f32: rows 0-63 ctx, row 64 denominator
    ctxA = consts.tile([128, DC, QF], F32)        # even heads
    ctxB = consts.tile([128, DC, QF], F32)        # odd heads
    out_attn = consts.tile([128, 2, D], F32)
    z3T = consts.tile([128, DC, KQ], BF16)

    # DRAM bounce for the pair AllToAll of ctx partials (bf16).
    # in[h] = this core's partials for q-half h (destined to rank h of the
    # pair); after A2A, out[0]+out[1] = pair-summed ctx for OUR q-half on
    # every rank, with uniform indexing.
    dram_cm = tc.tile_pool(name="dram", bufs=1, space="DRAM")
    dram = dram_cm.__enter__()
    cc_in = dram.tile([2, 2, 128, DC, KQ], BF16)   # [q-half, A/B, ...]
    cc_out = dram.tile([2, 2, 2, 128, DC, KQ], BF16)  # [rank, q-half, A/B]

    # ================= attention super-phase =================
    with (
        tc.tile_pool(name="wpool", bufs=2) as wpool,
        tc.tile_pool(name="zpool", bufs=2) as zpool,
        tc.tile_pool(name="ktp", bufs=2) as ktp,
        tc.tile_pool(name="vp", bufs=2) as vp,
        tc.tile_pool(name="scratch", bufs=2) as scr,
        tc.tile_pool(name="expp", bufs=3) as expp,
        tc.tile_pool(name="rowk", bufs=2) as rowk,
        tc.tile_pool(name="rkp", bufs=2) as rkp,
        tc.tile_pool(name="psB", bufs=4, space="PSUM") as psB,
        tc.tile_pool(name="psC", bufs=1, space="PSUM") as psC,
        tc.tile_pool(name="psR", bufs=2, space="PSUM") as psR,
    ):
        # ---------- helpers ----------
        def ln_stats(x_sb, ncols, t, pool):
            ps_s = psR.tile([1, ncols], F32, tag="psR", name="ps_s" + t)
            ps_q = psR.tile([1, ncols], F32, tag="psR", name="ps_q" + t)
            for cc in range(DC):
                sq = scr.tile([128, ncols], BF16, tag="sq" + t)
                nc.vector.tensor_tensor(sq[:], x_sb[:, cc, :], x_sb[:, cc, :],
                                        ALU.mult)
                nc.tensor.matmul(ps_s[:], ones_bf[:, 0:1], x_sb[:, cc, :],
                                 start=(cc == 0), stop=(cc == DC - 1))
                nc.tensor.matmul(ps_q[:], ones_bf[:, 0:1], sq[:],
                                 start=(cc == 0), stop=(cc == DC - 1))
            st = pool.tile([1, 2, ncols], F32, tag="st" + t)
            mu, msq = st[:, 0, :], st[:, 1, :]
            nc.vector.tensor_scalar_mul(mu, ps_s[:], 1.0 / D)
            nc.vector.tensor_scalar_mul(msq, ps_q[:], 1.0 / D)
            bfr = pool.tile([1, 2, ncols], BF16, tag="bfr" + t)
            rln, mrow = bfr[:, 0, :], bfr[:, 1, :]
            # var = msq - mu^2 (mu^2 via the bf16 rln slot; |mu|<<1 so fine)
            nc.vector.tensor_tensor(rln, mu, mu, ALU.mult)
            nc.vector.tensor_tensor(msq, msq, rln, ALU.subtract)
            nc.scalar.activation(out=msq, in_=msq, func=ACTF.Ln,
                                 bias=c_ln_eps[0:1, 0:1], scale=1.0)
            nc.scalar.activation(out=rln, in_=msq, func=ACTF.Exp,
                                 bias=c_zero[0:1, 0:1], scale=-0.5)
            nc.vector.tensor_tensor(mrow, mu, rln, ALU.mult)
            nc.vector.tensor_scalar_mul(mrow, mrow, -1.0)
            return rln, mrow

        def normalize(x_sb, z_sb, rln, mrow, ncols, t):
            rb = scr.tile([128, 2, ncols], BF16, tag="rb" + t)
            nc.gpsimd.partition_broadcast(rb[:, 0, :], rln)
            nc.gpsimd.partition_broadcast(rb[:, 1, :], mrow)
            nc.vector.tensor_tensor(
                z_sb[:], x_sb[:],
                rb[:, 0, :].unsqueeze(1).to_broadcast([128, DC, ncols]),
                ALU.mult)
            nc.vector.tensor_tensor(
                z_sb[:], z_sb[:],
                rb[:, 1, :].unsqueeze(1).to_broadcast([128, DC, ncols]),
                ALU.add)

        # prefetch all pf blocks early; the sync ring then frees up for the
        # MLP weight stream which follows it in sync-engine program order
        pf_pre = []
        for j in range(min(2, NBLK)):
            pfj = zpool.tile([128, DC, NB], BF16, tag="pf")
            nc.sync.dma_start(pfj[:], io["pf"][j])
            pf_pre.append(pfj)

        # ---------- phase Q: qhT (512 queries) ----------
        with (
            tc.tile_pool(name="qph", bufs=1) as qph,
            tc.tile_pool(name="rowq", bufs=1) as rowq,
        ):
            wq_sb = wpool.tile([128, DC, D], BF16, tag="w")
            nc.sync.dma_start(wq_sb[:], io["wq"])
            qt_sb = qph.tile([128, DC, QF], BF16, tag="qt", name="qt")
            nc.sync.dma_start(qt_sb[:], io["qt"])

            rln_q, mrow_q = ln_stats(qt_sb, QF, "q", rowq)
            normalize(qt_sb, qt_sb, rln_q, mrow_q, QF, "q")

            qraw = qph.tile([128, DC, QF], BF16, tag="qraw")
            for dc in range(DC):
                pq = psB.tile([128, QF], F32, tag="ps512")
                for cc in range(DC):
                    nc.tensor.matmul(pq[:],
                                     wq_sb[:, cc, dc * 128:(dc + 1) * 128],
                                     qt_sb[:, cc, :], start=(cc == 0),
                                     stop=(cc == DC - 1))
                nc.vector.tensor_scalar_add(qraw[:, dc, :], pq[:],
                                            cq_sb[:, dc:dc + 1])
            psq = psR.tile([1, QF], F32, tag="psR")
            for dc in range(DC):
                sqq = scr.tile([128, QF], BF16, tag="sqq")
                nc.vector.tensor_tensor(sqq[:], qraw[:, dc, :], qraw[:, dc, :],
                                        ALU.mult)
                nc.tensor.matmul(psq[:], ones_bf[:, 0:1], sqq[:],
                                 start=(dc == 0), stop=(dc == DC - 1))
            rq = rowq.tile([1, QF], F32, tag="rq")
            nc.scalar.activation(out=rq[:], in_=psq[:], func=ACTF.Ln,
                                 bias=c_rms_eps[0:1, 0:1], scale=1.0 / D)
            nc.scalar.activation(out=rq[:], in_=rq[:], func=ACTF.Exp,
                                 bias=c_zero[0:1, 0:1], scale=-0.5)
            rq_bc = qph.tile([128, QF], F32, tag="rqbc")
            nc.gpsimd.partition_broadcast(rq_bc[:], rq[:])
            for dc in range(DC):
                nc.vector.scalar_tensor_tensor(
                    out=qhT[:, dc, :], in0=qraw[:, dc, :],
                    scalar=wqk_sb[:, dc:dc + 1], in1=rq_bc[:],
                    op0=ALU.mult, op1=ALU.mult)

        # ---------- main: block-pipelined projections + attention ----------
        wk_sb = wpool.tile([128, DC, D], BF16, tag="w")
        nc.sync.dma_start(wk_sb[:], io["wk"])
        wv_sb = wpool.tile([128, DC, D], BF16, tag="w")
        nc.sync.dma_start(wv_sb[:], io["wv"])

        def proj_block(j):
            if j < len(pf_pre):
                pf = pf_pre[j]
            else:
                pf = zpool.tile([128, DC, NB], BF16, tag="pf")
                nc.sync.dma_start(pf[:], io["pf"][j])
            rln, mrow = ln_stats(pf, NB, "kv", rowk)
            normalize(pf, pf, rln, mrow, NB, "kv")
            z = pf
            kT = ktp.tile([128, DC, NB], BF16, tag="kt")
            for dc in range(DC):
                pk = psB.tile([128, NB], F32, tag="ps512")
                for cc in range(DC):
                    nc.tensor.matmul(
                        pk[:], wk_sb[:, cc, dc * 128:(dc + 1) * 128],
                        z[:, cc, :], start=(cc == 0), stop=(cc == DC - 1))
                nc.vector.tensor_scalar_add(kT[:, dc, :], pk[:],
                                            ck_sb[:, dc:dc + 1])
            v_j = vp.tile([128, S, H * 65], BF16, tag="v")
            ones_dst = bass.AP(
                tensor=v_j[:].tensor, offset=v_j[:, 0, 64:65].offset,
                ap=[v_j[:].ap[0], [H * 65, S], [65, H]])
            nc.vector.tensor_copy(
                ones_dst,
                ones_bf[:, 0:1].unsqueeze(1).to_broadcast([128, S, H]))
            for s4 in range(S):
                for dh in range(2):
                    pv = psB.tile([128, 512], F32, tag="ps512")
                    for cc in range(DC):
                        nc.tensor.matmul(
                            pv[:], z[:, cc, s4 * 128:(s4 + 1) * 128],
                            wv_sb[:, cc, dh * 512:(dh + 1) * 512],
                            start=(cc == 0), stop=(cc == DC - 1))
                    dst = bass.AP(
                        tensor=v_j[:].tensor,
                        offset=v_j[:, s4, dh * 8 * 65:dh * 8 * 65 + 1].offset,
                        ap=[v_j[:].ap[0], [65, 8], [1, 64]])
                    nc.vector.tensor_copy(dst, pv[:])
            # rk = (1/8)*rsqrt(mean(k^2)+eps), then pre-scale kT by it
            rk_row = rkp.tile([1, NB], F32, tag="rk")
            prk = psR.tile([1, NB], F32, tag="psR")
            for dc in range(DC):
                sqk = scr.tile([128, NB], BF16, tag="sqk")
                nc.vector.tensor_tensor(sqk[:], kT[:, dc, :], kT[:, dc, :],
                                        ALU.mult)
                nc.tensor.matmul(prk[:], ones_bf[:, 0:1], sqk[:],
                                 start=(dc == 0), stop=(dc == DC - 1))
            nc.scalar.activation(out=rk_row[:], in_=prk[:], func=ACTF.Ln,
                                 bias=c_rms_eps[0:1, 0:1], scale=1.0 / D)
            nc.scalar.activation(out=rk_row[:], in_=rk_row[:], func=ACTF.Exp,
                                 bias=c_mln8[0:1, 0:1], scale=-0.5)
            rk_bc = rkp.tile([128, NB], F32, tag="rkbc")
            nc.gpsimd.partition_broadcast(rk_bc[:], rk_row[:])
            for dc in range(DC):
                nc.vector.tensor_tensor(kT[:, dc, :], kT[:, dc, :], rk_bc[:],
                                        ALU.mult)
            return kT, v_j

        def attn_block(j, kT, v_j):
            for hp in range(DC):
                pc = psC.tile([128, 2, QF], F32, tag="psC")
                for s4 in range(S):
                    pa = psB.tile([128, QF], F32, tag="ps512")
                    pb = psB.tile([128, QF], F32, tag="ps512")
                    nlo = s4 * 128
                    nc.tensor.matmul(
                        pa[:], kT[0:64, hp, nlo:nlo + 128],
                        qhT[0:64, hp, :], start=True, stop=True,
                        tile_position=(0, 0))
                    nc.tensor.matmul(
                        pb[:], kT[64:128, hp, nlo:nlo + 128],
                        qhT[64:128, hp, :], start=True, stop=True,
                        tile_position=(64, 0))
                    es = expp.tile([128, 2, QF], BF16, tag="es")
                    nc.scalar.activation(out=es[:, 0], in_=pa[:],
                                         func=ACTF.Exp, bias=negc[:],
                                         scale=1.0)
                    nc.scalar.activation(out=es[:, 1], in_=pb[:],
                                         func=ACTF.Exp, bias=negc[:],
                                         scale=1.0)
                    for hh in range(2):
                        h = 2 * hp + hh
                        nc.tensor.matmul(
                            pc[0:65, hh, :],
                            v_j[:, s4, h * 65:(h + 1) * 65],
                            es[:, hh, :],
                            start=(s4 == 0), stop=(s4 == S - 1))
                for hh, ctx_acc in ((0, ctxA), (1, ctxB)):
                    if j == 0:
                        nc.vector.tensor_copy(ctx_acc[0:65, hp, :],
                                              pc[0:65, hh, :])
                    else:
                        nc.vector.tensor_tensor(ctx_acc[0:65, hp, :],
                                                ctx_acc[0:65, hp, :],
                                                pc[0:65, hh, :], ALU.add)

        pending = None
        for j in range(NBLK):
            cur = proj_block(j)
            if pending is not None:
                attn_block(*pending)
            pending = (j, *cur)
        attn_block(*pending)

        # ---------- pair AllGather of ctx partials ----------
        for qc in range(2):
            ql = qc * KQ
            nc.gpsimd.dma_start(cc_in[qc, 0], ctxA[:, :, ql:ql + KQ])
            nc.gpsimd.dma_start(cc_in[qc, 1], ctxB[:, :, ql:ql + KQ])
        nc.gpsimd.collective_compute(
            "AllGather", ALU.bypass, replica_groups=RG,
            ins=[cc_in.opt()], outs=[cc_out.opt()],
        )

    # ---------- normalize ctx, Wo projection, residual, LN3 ----------
    with (
        tc.tile_pool(name="late", bufs=1) as latep,
        tc.tile_pool(name="lscr", bufs=2) as lscr,
        tc.tile_pool(name="psW", bufs=2, space="PSUM") as psW,
        tc.tile_pool(name="psT", bufs=2, space="PSUM") as psT,
    ):
        bo_bc = latep.tile([128, D], F32)
        bcast_row(bo_bc[:], io["bo_row"])
        qres_sb = latep.tile([128, 2, D], F32)
        nc.scalar.dma_start(qres_sb[:], io["qres"])
        wo_sb = latep.tile([128, DC, D], BF16)
        nc.scalar.dma_start(wo_sb[:], io["wo"])

        hsel = latep.tile([128, 1], F32)
        nc.sync.dma_start(hsel[:], io["hsel"])
        ctxAs = latep.tile([128, DC, KQ], F32)
        ctxBs = latep.tile([128, DC, KQ], F32)
        parts = latep.tile([128, 2, 2, 2, DC, KQ], BF16)
        for r in range(2):
            for sh in range(2):
                for ab in range(2):
                    nc.gpsimd.dma_start(parts[:, r, sh, ab],
                                        cc_out[r, sh, ab])
        # ssum[h] = rank0 + rank1 partials for q-half h; then pick own half:
        # ctx = ssum[1] + hsel*(ssum[0]-ssum[1])   (hsel = 1 - my_half)
        ssum = latep.tile([128, 2, 2, DC, KQ], F32)
        nc.vector.tensor_tensor(ssum[:], parts[:, 0], parts[:, 1], ALU.add)
        nc.vector.tensor_tensor(ssum[:, 0], ssum[:, 0], ssum[:, 1],
                                ALU.subtract)
        nc.vector.scalar_tensor_tensor(
            out=ctxAs[:], in0=ssum[:, 0, 0], scalar=hsel[:, 0:1],
            in1=ssum[:, 1, 0], op0=ALU.mult, op1=ALU.add)
        nc.vector.scalar_tensor_tensor(
            out=ctxBs[:], in0=ssum[:, 0, 1], scalar=hsel[:, 0:1],
            in1=ssum[:, 1, 1], op0=ALU.mult, op1=ALU.add)

        cxh = latep.tile([128, DC, KQ], BF16)
        dinv = latep.tile([1, 2, DC, KQ], F32)
        nc.vector.reciprocal(dinv[:, 0], ctxAs[64:65, :, :])
        nc.vector.reciprocal(dinv[:, 1], ctxBs[64:65, :, :])
        for h in range(H):
            hp, hh = h // 2, h % 2
            ctx_acc = ctxAs if hh == 0 else ctxBs
            rb = lscr.tile([128, KQ], F32, tag="recb")
            nc.gpsimd.partition_broadcast(rb[:], dinv[:, hh, hp, :])
            lo = hh * 64
            nc.vector.tensor_tensor(cxh[lo:lo + 64, hp, :],
                                    ctx_acc[0:64, hp, :],
                                    rb[0:64, :], ALU.mult)

        for s in range(2):
            for dh in range(2):
                po = psW.tile([128, 512], F32, tag="psW")
                for dc in range(DC):
                    nc.tensor.matmul(po[:], cxh[:, dc, s * 128:(s + 1) * 128],
                                     wo_sb[:, dc, dh * 512:(dh + 1) * 512],
                                     start=(dc == 0), stop=(dc == DC - 1))
                nc.vector.tensor_tensor(
                    out_attn[:, s, dh * 512:(dh + 1) * 512], po[:],
                    bo_bc[:, dh * 512:(dh + 1) * 512], ALU.add)
            nc.vector.tensor_tensor(out_attn[:, s, :], out_attn[:, s, :],
                                    qres_sb[:, s, :], ALU.add)

        # LN3 + transpose to z3T
        for s in range(2):
            stats = lscr.tile([128, 2, 6], F32, tag="bn3")
            nc.vector.bn_stats(stats[:, 0, :], out_attn[:, s, 0:512])
            nc.vector.bn_stats(stats[:, 1, :], out_attn[:, s, 512:1024])
            mv = lscr.tile([128, 2], F32, tag="mv3")
            nc.vector.bn_aggr(mv[:], stats[:])
            rstd = lscr.tile([128, 1], F32, tag="rstd3")
            nc.scalar.activation(out=rstd[:], in_=mv[:, 1:2], func=ACTF.Ln,
                                 bias=c_ln_eps[:], scale=1.0)
            nc.scalar.activation(out=rstd[:], in_=rstd[:], func=ACTF.Exp,
                                 bias=c_zero[:], scale=-0.5)
            nbias = lscr.tile([128, 1], F32, tag="nb3")
            nc.vector.tensor_tensor(nbias[:], mv[:, 0:1], rstd[:], ALU.mult)
            nc.vector.tensor_scalar_mul(nbias[:], nbias[:], -1.0)
            z3 = lscr.tile([128, 2, 512], BF16, tag="z3")
            for half in range(2):
                nc.scalar.activation(
                    out=z3[:, half, :],
                    in_=out_attn[:, s, half * 512:(half + 1) * 512],
                    func=ACTF.Identity, bias=nbias[:], scale=rstd[:])
            for dc in range(DC):
                pt = psT.tile([128, 128], BF16, tag="psT")
                nc.tensor.transpose(
                    pt[:], z3[:, dc // 4, (dc % 4) * 128:(dc % 4 + 1) * 128],
                    ident[:])
                nc.vector.tensor_copy(z3T[:, dc, s * 128:(s + 1) * 128], pt[:])

        if DEBUG:
            nc.sync.dma_start(io["dbg_ctx"], cxh[:])
            nc.sync.dma_start(io["dbg_attn"], out_attn[:])

    # ================= MLP super-phase =================
    with (
        tc.tile_pool(name="mw", bufs=4) as mw,
        tc.tile_pool(name="gt", bufs=2) as gtp,
        tc.tile_pool(name="mrow2", bufs=1) as mrow2,
        tc.tile_pool(name="psH", bufs=2, space="PSUM") as psH,
        tc.tile_pool(name="psO", bufs=1, space="PSUM") as psO,
    ):
        b2_bc = mrow2.tile([128, D], F32)
        nc.gpsimd.dma_start(
            out=b2_bc[:],
            in_=bass.AP(tensor=io["b2_row"].tensor, offset=io["b2_row"].offset,
                        ap=[[0, 128], io["b2_row"].ap[1]]),
        )
        pouts = {}
        for s in range(2):
            for dh in range(2):
                pouts[(s, dh)] = psO.tile([128, 512], F32, tag=f"po{s}{dh}",
                                          name=f"po{s}{dh}")
        nfc = FF // FC  # 8
        for fc in range(nfc):
            w1c = mw.tile([128, DC, FC], BF16, tag="w1")
            nc.sync.dma_start(w1c[:], io["w1"][fc])
            w2c = mw.tile([128, FC // 128, D], BF16, tag="w2")
            nc.sync.dma_start(w2c[:], io["w2"][fc])
            gt = gtp.tile([128, FC // 128, KQ], BF16, tag="gt")
            for fp in range(2):
                ph = psH.tile([128, 2, KQ], F32, tag="psH")
                for fi in range(2):
                    fs = fp * 2 + fi
                    for cc in range(DC):
                        nc.tensor.matmul(
                            ph[:, fi, :], w1c[:, cc, fs * 128:(fs + 1) * 128],
                            z3T[:, cc, :], start=(cc == 0), stop=(cc == DC - 1))
                fidx0 = fc * (FC // 128) + fp * 2
                for fi in range(2):
                    nc.scalar.activation(
                        out=gt[:, fp * 2 + fi, :], in_=ph[:, fi, :],
                        func=ACTF.Gelu,
                        bias=c1_sb[:, fidx0 + fi:fidx0 + fi + 1], scale=1.0)
            for s in range(2):
                for dh in range(2):
                    for fs in range(FC // 128):
                        nc.tensor.matmul(
                            pouts[(s, dh)][:], gt[:, fs, s * 128:(s + 1) * 128],
                            w2c[:, fs, dh * 512:(dh + 1) * 512],
                            start=(fc == 0 and fs == 0),
                            stop=(fc == nfc - 1 and fs == FC // 128 - 1))

        for s in range(2):
            for dh in range(2):
                sl = slice(dh * 512, (dh + 1) * 512)
                nc.vector.tensor_tensor(out_attn[:, s, sl], pouts[(s, dh)][:],
                                        out_attn[:, s, sl], ALU.add)
            nc.vector.tensor_tensor(out_attn[:, s, :], out_attn[:, s, :],
                                    b2_bc[:], ALU.add)
        nc.sync.dma_start(io["out"], out_attn[:])

    dram_cm.__exit__(None, None, None)


def build():
    nc = bacc.Bacc("TRN2", target_bir_lowering=False, debug=False,
                   num_devices=8)
    io = {}
    io["pf"] = [
        nc.dram_tensor(f"pf{j}", [128, DC, NB], BF16, kind="ExternalInput").ap()
        for j in range(NBLK)
    ]
    io["qt"] = nc.dram_tensor("qt", [128, DC, QF], BF16, kind="ExternalInput").ap()
    io["qres"] = nc.dram_tensor("qres", [128, 2, D], F32, kind="ExternalInput").ap()
    for w in ["wq", "wk", "wv", "wo"]:
        io[w] = nc.dram_tensor(w, [128, DC, D], BF16, kind="ExternalInput").ap()
    io["w1"] = [
        nc.dram_tensor(f"w1_{i}", [128, DC, FC], BF16, kind="ExternalInput").ap()
        for i in range(FF // FC)
    ]
    io["w2"] = [
        nc.dram_tensor(f"w2_{i}", [128, FC // 128, D], BF16,
                       kind="ExternalInput").ap()
        for i in range(FF // FC)
    ]
    io["ck"] = nc.dram_tensor("ck", [128, DC], F32, kind="ExternalInput").ap()
    io["cq"] = nc.dram_tensor("cq", [128, DC], F32, kind="ExternalInput").ap()
    io["wqk"] = nc.dram_tensor("wqk", [128, DC], F32, kind="ExternalInput").ap()
    io["c1"] = nc.dram_tensor("c1", [128, FF // 128], F32, kind="ExternalInput").ap()
    io["bo_row"] = nc.dram_tensor("bo_row", [1, D], F32, kind="ExternalInput").ap()
    io["hsel"] = nc.dram_tensor("hsel", [128, 1], F32, kind="ExternalInput").ap()
    io["b2_row"] = nc.dram_tensor("b2_row", [1, D], F32, kind="ExternalInput").ap()
    io["out"] = nc.dram_tensor("out", [128, 2, D], F32, kind="ExternalOutput").ap()
    if DEBUG:
        for name, shape, dt in [
            ("dbg_ctx", [128, DC, KQ], BF16),
            ("dbg_attn", [128, 2, D], F32),
        ]:
            io[name] = nc.dram_tensor(name, shape, dt, kind="ExternalOutput").ap()

    with tile.TileContext(nc) as tc:
        with tc.tile_pool(name="consts", bufs=1) as consts:
            _emit(nc, tc, io, consts)
    nc.compile()
    return nc


def prep_core_inputs(inputs, core):
    """Host-side fold + shard + relayout for one core."""
    b, half = core // 2, core % 2
    f32 = np.float32
    qt_full = np.asarray(inputs["query_tokens"], f32)
    pf_full = np.asarray(inputs["point_features"], f32)
    Wq = np.asarray(inputs["Wq"], f32)
    Wk = np.asarray(inputs["Wk"], f32)
    Wv = np.asarray(inputs["Wv"], f32)
    Wo = np.asarray(inputs["Wo"], f32)
    W1 = np.asarray(inputs["W1"], f32)
    W2 = np.asarray(inputs["W2"], f32)
    g_q, b_q = np.asarray(inputs["ln_q_g"], f32), np.asarray(inputs["ln_q_b"], f32)
    g_kv, b_kv = np.asarray(inputs["ln_kv_g"], f32), np.asarray(inputs["ln_kv_b"], f32)
    g_m, b_m = np.asarray(inputs["ln_mlp_g"], f32), np.asarray(inputs["ln_mlp_b"], f32)

    Wqp = g_q[:, None] * Wq
    c_q = b_q @ Wq + np.asarray(inputs["bq"], f32)
    Wkp = g_kv[:, None] * Wk
    c_k = b_kv @ Wk + np.asarray(inputs["bk"], f32)
    Wvp = g_kv[:, None] * Wv
    c_v = b_kv @ Wv + np.asarray(inputs["bv"], f32)
    W1p = g_m[:, None] * W1
    c_1 = b_m @ W1 + np.asarray(inputs["b1"], f32)
    wqk = (np.asarray(inputs["rms_q_w"], f32) * np.asarray(inputs["rms_k_w"], f32))
    bo_f = np.asarray(inputs["bo"], f32) + c_v @ Wo   # fold v-bias into bo

    q_res = qt_full[b, half * KQ:(half + 1) * KQ]          # own 256 rows
    pfT = np.ascontiguousarray(pf_full[b].T)               # [D, N]
    qT = np.ascontiguousarray(qt_full[b].T)                # [D, 512] all queries

    def part_major(w, dt=NPBF):  # [D, X] -> [128, D//128, X]
        return np.ascontiguousarray(
            w.reshape(DC, 128, -1).transpose(1, 0, 2).astype(dt))

    m = {}
    # this core's kv half: blocks [half*NBLK, half*NBLK+NBLK)
    pf_dev = pfT.reshape(DC, 128, N // NB, NB).transpose(2, 1, 0, 3)
    for j in range(NBLK):
        m[f"pf{j}"] = np.ascontiguousarray(pf_dev[half * NBLK + j].astype(NPBF))
    m["qt"] = part_major(qT)
    m["qres"] = np.ascontiguousarray(q_res.reshape(2, 128, D).transpose(1, 0, 2))
    m["wq"] = part_major(Wqp)
    m["wk"] = part_major(Wkp)
    m["wv"] = part_major(Wvp)
    m["wo"] = part_major(Wo)
    w1_dev = part_major(W1p)                               # [128, 8, 4096]
    for i in range(FF // FC):
        m[f"w1_{i}"] = np.ascontiguousarray(w1_dev[:, :, i * FC:(i + 1) * FC])
    w2_dev = np.ascontiguousarray(
        W2.reshape(FF // 128, 128, D).transpose(1, 0, 2).astype(NPBF))
    for i in range(FF // FC):
        m[f"w2_{i}"] = np.ascontiguousarray(
            w2_dev[:, i * (FC // 128):(i + 1) * (FC // 128), :])
    m["ck"] = np.ascontiguousarray(c_k.reshape(DC, 128).T)
    m["cq"] = np.ascontiguousarray(c_q.reshape(DC, 128).T)
    m["wqk"] = np.ascontiguousarray(wqk.reshape(DC, 128).T)
    m["c1"] = np.ascontiguousarray(c_1.reshape(FF // 128, 128).T)
    m["bo_row"] = bo_f.reshape(1, D)
    m["hsel"] = np.full((128, 1), 1.0 - half, np.float32)
    m["b2_row"] = np.asarray(inputs["b2"], f32).reshape(1, D)
    return m


_NC_CACHE = None


def run_cores(inputs, **kw):
    global _NC_CACHE
    if _NC_CACHE is None:
        _NC_CACHE = build()
    in_maps = [prep_core_inputs(inputs, c) for c in range(8)]
    return run_bass_kernel_spmd(_NC_CACHE, in_maps, core_ids=list(range(8)), **kw)


def kernel(**inputs):
    res = run_cores(inputs)
    B, K = 4, 512
    out = np.zeros((B, K, D), np.float32)
    for c in range(8):
        b, half = c // 2, c % 2
        o = res.results[c]["out"]                          # [128, 2, 1024]
        out[b, half * KQ:(half + 1) * KQ] = o.transpose(1, 0, 2).reshape(KQ, D)
    return out


# revision 27
# speedup vs baseline: 1.3058x; 1.1403x over previous
"""Trainium2 Bass kernel for CrossAttentionBlock (nn_CrossAttentionBlock_12317966205103).

Sharding (v3b): 8 cores = 4 batches x 2 KV-halves. Each core computes
LN/K/V for its 2048 kv rows and attention of ALL 512 queries against
them, producing partial softmax numerators + denominators. A pair
ReduceScatter (bf16) sums the partials and hands each core its 256
output rows, which then run Wo/LN3/MLP locally (no other comms).

Device math (per core):
  z    = LN(point_features^T)            [D, 2048] bf16 (stats via ones-matmuls)
  kT   = Wk'^T z + c_k, then *= rk/8     [D, 2048] bf16
  v    = z^T Wv' stored 65-col head groups + ones col (denominator)
  qhT  = rms/weight-folded query proj    [D, 512] bf16
  per (head-pair, q-chunk, block): sT = kT_h^T qhT_h; e = exp(sT - 8);
    ctx_aug += [v_h | 1]^T e  (PSUM per block, SBUF f32 across blocks)
  ReduceScatter(pair) of [2, ctx] -> own 256-q ctx summed
  out_attn = (ctx/den)^T Wo + bo' + residual;  LN3;  gelu MLP;  sum.

bf16 matmul operands everywhere (FWL hides LDWEIGHTS); rsqrt via
exp(-0.5*ln(x)) so only the natural_log_exp table set is live; kT
pre-scaled by rk so exp ACTs batch [128,512]; v-bias folded host-side
into bo' = bo + c_v @ Wo.
"""

import os

import ml_dtypes
import numpy as np

import concourse.bass as bass
import concourse.tile as tile
from concourse import bacc, mybir
from concourse.bass_utils import run_bass_kernel_spmd
from concourse.masks import make_identity

F32 = mybir.dt.float32
BF16 = mybir.dt.bfloat16
NPBF = ml_dtypes.bfloat16
ALU = mybir.AluOpType
ACTF = mybir.ActivationFunctionType

D = 1024
N = 4096
NL = N // 2       # kv rows per core
QF = 512          # queries per core (attention)
KQ = 256          # output query rows per core
H = 16
HD = 64
FF = 4096         # mlp hidden
NB = 512          # n-block size
NBLK = NL // NB   # 4
S = NB // 128     # 4 n-subchunks per block
DC = D // 128     # 8 d-chunks
FC = 512          # mlp f-chunk
NEG_C = -8.0      # softmax stability shift (scores observed in [-8, 8])
LN8 = 2.0794415416798357
RG = [[0, 1], [2, 3], [4, 5], [6, 7]]

DEBUG = os.environ.get("BASSK_DEBUG", "0") == "1"

LN_EPS = 1e-5
RMS_EPS = 1e-6


def _emit(nc, tc, io, consts):
    # ---------- whole-program constants / survivors ----------
    ident = consts.tile([128, 128], BF16)
    make_identity(nc, ident[:])

    ones_f32 = consts.tile([128, 2], F32)
    nc.vector.memset(ones_f32[:], 1.0)
    ones_bf = consts.tile([128, 2], BF16)
    nc.vector.tensor_copy(ones_bf[:], ones_f32[:])

    negc = consts.tile([128, 1], F32)
    nc.vector.memset(negc[:], NEG_C)
    c_zero = consts.tile([128, 1], F32)
    nc.vector.memset(c_zero[:], 0.0)
    c_ln_eps = consts.tile([128, 1], F32)
    nc.vector.memset(c_ln_eps[:], LN_EPS)
    c_rms_eps = consts.tile([128, 1], F32)
    nc.vector.memset(c_rms_eps[:], RMS_EPS)
    c_mln8 = consts.tile([128, 1], F32)
    nc.vector.memset(c_mln8[:], -LN8)

    ck_sb = consts.tile([128, DC], F32)
    nc.sync.dma_start(ck_sb[:], io["ck"])
    cq_sb = consts.tile([128, DC], F32)
    nc.sync.dma_start(cq_sb[:], io["cq"])
    wqk_sb = consts.tile([128, DC], F32)
    nc.sync.dma_start(wqk_sb[:], io["wqk"])
    c1_sb = consts.tile([128, FF // 128], F32)
    nc.sync.dma_start(c1_sb[:], io["c1"])

    def bcast_row(dst, src_ap):
        nc.gpsimd.dma_start(
            out=dst,
            in_=bass.AP(tensor=src_ap.tensor, offset=src_ap.offset,
                        ap=[[0, 128], src_ap.ap[1]]),
        )

    qhT = consts.tile([128, DC, QF], BF16)        # \hat q ^T (512 q)
    # partial ctx accumulators f32: rows 0-63 ctx, row 64 denominator
    ctxA = consts.tile([128, DC, QF], F32)        # even heads
    ctxB = consts.tile([128, DC, QF], F32)        # odd heads
    out_attn = consts.tile([128, 2, D], F32)
    z3T = consts.tile([128, DC, KQ], BF16)

    # DRAM bounce for the pair AllToAll of ctx partials (bf16).
    # in[h] = this core's partials for q-half h (destined to rank h of the
    # pair); after A2A, out[0]+out[1] = pair-summed ctx for OUR q-half on
    # every rank, with uniform indexing.
    dram_cm = tc.tile_pool(name="dram", bufs=1, space="DRAM")
    dram = dram_cm.__enter__()
    cc_in = dram.tile([2, 128, DC, KQ], BF16)      # partner half [A/B, ...]
    cc_out = dram.tile([2, 2, 128, DC, KQ], BF16)  # [rank, A/B, ...]

    # ================= attention super-phase =================
    with (
        tc.tile_pool(name="wpool", bufs=2) as wpool,
        tc.tile_pool(name="zpool", bufs=3) as zpool,
        tc.tile_pool(name="ktp", bufs=2) as ktp,
        tc.tile_pool(name="vp", bufs=2) as vp,
        tc.tile_pool(name="scratch", bufs=2) as scr,
        tc.tile_pool(name="expp", bufs=3) as expp,
        tc.tile_pool(name="rowk", bufs=2) as rowk,
        tc.tile_pool(name="rkp", bufs=2) as rkp,
        tc.tile_pool(name="psB", bufs=4, space="PSUM") as psB,
        tc.tile_pool(name="psC", bufs=1, space="PSUM") as psC,
        tc.tile_pool(name="psR", bufs=2, space="PSUM") as psR,
    ):
        # ---------- helpers ----------
        def ln_stats(x_sb, ncols, t, pool):
            ps_s = psR.tile([1, ncols], F32, tag="psR", name="ps_s" + t)
            ps_q = psR.tile([1, ncols], F32, tag="psR", name="ps_q" + t)
            for cc in range(DC):
                sq = scr.tile([128, ncols], BF16, tag="sq" + t)
                nc.vector.tensor_tensor(sq[:], x_sb[:, cc, :], x_sb[:, cc, :],
                                        ALU.mult)
                nc.tensor.matmul(ps_s[:], ones_bf[:, 0:1], x_sb[:, cc, :],
                                 start=(cc == 0), stop=(cc == DC - 1))
                nc.tensor.matmul(ps_q[:], ones_bf[:, 0:1], sq[:],
                                 start=(cc == 0), stop=(cc == DC - 1))
            st = pool.tile([1, 2, ncols], F32, tag="st" + t)
            mu, msq = st[:, 0, :], st[:, 1, :]
            nc.vector.tensor_scalar_mul(mu, ps_s[:], 1.0 / D)
            nc.vector.tensor_scalar_mul(msq, ps_q[:], 1.0 / D)
            bfr = pool.tile([1, 2, ncols], BF16, tag="bfr" + t)
            rln, mrow = bfr[:, 0, :], bfr[:, 1, :]
            # var = msq - mu^2 (mu^2 via the bf16 rln slot; |mu|<<1 so fine)
            nc.vector.tensor_tensor(rln, mu, mu, ALU.mult)
            nc.vector.tensor_tensor(msq, msq, rln, ALU.subtract)
            nc.scalar.activation(out=msq, in_=msq, func=ACTF.Ln,
                                 bias=c_ln_eps[0:1, 0:1], scale=1.0)
            nc.scalar.activation(out=rln, in_=msq, func=ACTF.Exp,
                                 bias=c_zero[0:1, 0:1], scale=-0.5)
            nc.vector.tensor_tensor(mrow, mu, rln, ALU.mult)
            nc.vector.tensor_scalar_mul(mrow, mrow, -1.0)
            return rln, mrow

        def normalize(x_sb, z_sb, rln, mrow, ncols, t):
            rb = scr.tile([128, 2, ncols], BF16, tag="rb" + t)
            nc.gpsimd.partition_broadcast(rb[:, 0, :], rln)
            nc.gpsimd.partition_broadcast(rb[:, 1, :], mrow)
            nc.vector.tensor_tensor(
                z_sb[:], x_sb[:],
                rb[:, 0, :].unsqueeze(1).to_broadcast([128, DC, ncols]),
                ALU.mult)
            nc.vector.tensor_tensor(
                z_sb[:], z_sb[:],
                rb[:, 1, :].unsqueeze(1).to_broadcast([128, DC, ncols]),
                ALU.add)

        # prefetch all pf blocks early; the sync ring then frees up for the
        # MLP weight stream which follows it in sync-engine program order
        pf_pre = []
        for j in range(min(2, NBLK)):
            pfj = zpool.tile([128, DC, NB], BF16, tag="pf")
            nc.sync.dma_start(pfj[:], io["pf"][j])
            pf_pre.append(pfj)

        # ---------- phase Q: qhT (512 queries) ----------
        with (
            tc.tile_pool(name="qph", bufs=1) as qph,
            tc.tile_pool(name="rowq", bufs=1) as rowq,
        ):
            qt_sb = qph.tile([128, DC, QF], BF16, tag="qt", name="qt")
            nc.sync.dma_start(qt_sb[:], io["qt"])
            wq_sb = wpool.tile([128, DC, D], BF16, tag="w")
            nc.sync.dma_start(wq_sb[:], io["wq"])

            rln_q, mrow_q = ln_stats(qt_sb, QF, "q", rowq)
            normalize(qt_sb, qt_sb, rln_q, mrow_q, QF, "q")

            qraw = qph.tile([128, DC, QF], BF16, tag="qraw")
            for dc in range(DC):
                pq = psB.tile([128, QF], F32, tag="ps512")
                for cc in range(DC):
                    nc.tensor.matmul(pq[:],
                                     wq_sb[:, cc, dc * 128:(dc + 1) * 128],
                                     qt_sb[:, cc, :], start=(cc == 0),
                                     stop=(cc == DC - 1))
                nc.vector.tensor_scalar_add(qraw[:, dc, :], pq[:],
                                            cq_sb[:, dc:dc + 1])
            psq = psR.tile([1, QF], F32, tag="psR")
            for dc in range(DC):
                sqq = scr.tile([128, QF], BF16, tag="sqq")
                nc.vector.tensor_tensor(sqq[:], qraw[:, dc, :], qraw[:, dc, :],
                                        ALU.mult)
                nc.tensor.matmul(psq[:], ones_bf[:, 0:1], sqq[:],
                                 start=(dc == 0), stop=(dc == DC - 1))
            rq = rowq.tile([1, QF], F32, tag="rq")
            nc.scalar.activation(out=rq[:], in_=psq[:], func=ACTF.Ln,
                                 bias=c_rms_eps[0:1, 0:1], scale=1.0 / D)
            nc.scalar.activation(out=rq[:], in_=rq[:], func=ACTF.Exp,
                                 bias=c_zero[0:1, 0:1], scale=-0.5)
            rq_bc = qph.tile([128, QF], F32, tag="rqbc")
            nc.gpsimd.partition_broadcast(rq_bc[:], rq[:])
            for dc in range(DC):
                nc.vector.scalar_tensor_tensor(
                    out=qhT[:, dc, :], in0=qraw[:, dc, :],
                    scalar=wqk_sb[:, dc:dc + 1], in1=rq_bc[:],
                    op0=ALU.mult, op1=ALU.mult)

        # ---------- main: block-pipelined projections + attention ----------
        wk_sb = wpool.tile([128, DC, D], BF16, tag="w")
        nc.sync.dma_start(wk_sb[:], io["wk"])
        wv_sb = wpool.tile([128, DC, D], BF16, tag="w")
        nc.sync.dma_start(wv_sb[:], io["wv"])

        def prep_z(j):
            if j < len(pf_pre):
                pf = pf_pre[j]
            else:
                pf = zpool.tile([128, DC, NB], BF16, tag="pf")
                nc.sync.dma_start(pf[:], io["pf"][j])
            rln, mrow = ln_stats(pf, NB, "kv", rowk)
            normalize(pf, pf, rln, mrow, NB, "kv")
            return pf

        def proj_block(j, z, z_prep):
            kT = ktp.tile([128, DC, NB], BF16, tag="kt")
            for dc in range(DC):
                pk = psB.tile([128, NB], F32, tag="ps512")
                for cc in range(DC):
                    nc.tensor.matmul(
                        pk[:], wk_sb[:, cc, dc * 128:(dc + 1) * 128],
                        z[:, cc, :], start=(cc == 0), stop=(cc == DC - 1))
                nc.vector.tensor_scalar_add(kT[:, dc, :], pk[:],
                                            ck_sb[:, dc:dc + 1])
            z_next = z_prep()
            v_j = vp.tile([128, S, H * 65], BF16, tag="v")
            ones_dst = bass.AP(
                tensor=v_j[:].tensor, offset=v_j[:, 0, 64:65].offset,
                ap=[v_j[:].ap[0], [H * 65, S], [65, H]])
            nc.vector.tensor_copy(
                ones_dst,
                ones_bf[:, 0:1].unsqueeze(1).to_broadcast([128, S, H]))
            for s4 in range(S):
                for dh in range(2):
                    pv = psB.tile([128, 512], F32, tag="ps512")
                    for cc in range(DC):
                        nc.tensor.matmul(
                            pv[:], z[:, cc, s4 * 128:(s4 + 1) * 128],
                            wv_sb[:, cc, dh * 512:(dh + 1) * 512],
                            start=(cc == 0), stop=(cc == DC - 1))
                    dst = bass.AP(
                        tensor=v_j[:].tensor,
                        offset=v_j[:, s4, dh * 8 * 65:dh * 8 * 65 + 1].offset,
                        ap=[v_j[:].ap[0], [65, 8], [1, 64]])
                    nc.vector.tensor_copy(dst, pv[:])
            # rk = (1/8)*rsqrt(mean(k^2)+eps), then pre-scale kT by it
            rk_row = rkp.tile([1, NB], F32, tag="rk")
            prk = psR.tile([1, NB], F32, tag="psR")
            for dc in range(DC):
                sqk = scr.tile([128, NB], BF16, tag="sqk")
                nc.vector.tensor_tensor(sqk[:], kT[:, dc, :], kT[:, dc, :],
                                        ALU.mult)
                nc.tensor.matmul(prk[:], ones_bf[:, 0:1], sqk[:],
                                 start=(dc == 0), stop=(dc == DC - 1))
            nc.scalar.activation(out=rk_row[:], in_=prk[:], func=ACTF.Ln,
                                 bias=c_rms_eps[0:1, 0:1], scale=1.0 / D)
            nc.scalar.activation(out=rk_row[:], in_=rk_row[:], func=ACTF.Exp,
                                 bias=c_mln8[0:1, 0:1], scale=-0.5)
            rk_bc = rkp.tile([128, NB], F32, tag="rkbc")
            nc.gpsimd.partition_broadcast(rk_bc[:], rk_row[:])
            for dc in range(DC):
                nc.vector.tensor_tensor(kT[:, dc, :], kT[:, dc, :], rk_bc[:],
                                        ALU.mult)
            return kT, v_j, z_next

        def attn_block(j, kT, v_j):
            for hp in range(DC):
                pc = psC.tile([128, 2, QF], F32, tag="psC")
                for s4 in range(S):
                    pa = psB.tile([128, QF], F32, tag="ps512")
                    pb = psB.tile([128, QF], F32, tag="ps512")
                    nlo = s4 * 128
                    nc.tensor.matmul(
                        pa[:], kT[0:64, hp, nlo:nlo + 128],
                        qhT[0:64, hp, :], start=True, stop=True,
                        tile_position=(0, 0))
                    nc.tensor.matmul(
                        pb[:], kT[64:128, hp, nlo:nlo + 128],
                        qhT[64:128, hp, :], start=True, stop=True,
                        tile_position=(64, 0))
                    es = expp.tile([128, 2, QF], BF16, tag="es")
                    nc.scalar.activation(out=es[:, 0], in_=pa[:],
                                         func=ACTF.Exp, bias=negc[:],
                                         scale=1.0)
                    nc.scalar.activation(out=es[:, 1], in_=pb[:],
                                         func=ACTF.Exp, bias=negc[:],
                                         scale=1.0)
                    for hh in range(2):
                        h = 2 * hp + hh
                        nc.tensor.matmul(
                            pc[0:65, hh, :],
                            v_j[:, s4, h * 65:(h + 1) * 65],
                            es[:, hh, :],
                            start=(s4 == 0), stop=(s4 == S - 1))
                for hh, ctx_acc in ((0, ctxA), (1, ctxB)):
                    if j == 0:
                        nc.vector.tensor_copy(ctx_acc[0:65, hp, :],
                                              pc[0:65, hh, :])
                    else:
                        nc.vector.tensor_tensor(ctx_acc[0:65, hp, :],
                                                ctx_acc[0:65, hp, :],
                                                pc[0:65, hh, :], ALU.add)

        pending = None
        z_cur = prep_z(0)
        for j in range(NBLK):
            prep = ((lambda jn=j + 1: prep_z(jn)) if j + 1 < NBLK
                    else (lambda: None))
            kT_j, v_j, z_cur_next = proj_block(j, z_cur, prep)
            if pending is not None:
                attn_block(*pending)
            pending = (j, kT_j, v_j)
            z_cur = z_cur_next
        attn_block(*pending)

        # ---------- pair AllGather of the partner-destined ctx half ----------
        # host orders queries so each core's own half is local cols 0:KQ;
        # cols KQ:2KQ belong to the partner.
        nc.gpsimd.dma_start(cc_in[0], ctxA[:, :, KQ:2 * KQ])
        nc.gpsimd.dma_start(cc_in[1], ctxB[:, :, KQ:2 * KQ])
        nc.gpsimd.collective_compute(
            "AllGather", ALU.bypass, replica_groups=RG,
            ins=[cc_in.opt()], outs=[cc_out.opt()],
        )

    # ---------- normalize ctx, Wo projection, residual, LN3 ----------
    with (
        tc.tile_pool(name="late", bufs=1) as latep,
        tc.tile_pool(name="lscr", bufs=2) as lscr,
        tc.tile_pool(name="psW", bufs=2, space="PSUM") as psW,
        tc.tile_pool(name="psT", bufs=2, space="PSUM") as psT,
    ):
        bo_bc = latep.tile([128, D], F32)
        bcast_row(bo_bc[:], io["bo_row"])
        qres_sb = latep.tile([128, 2, D], F32)
        nc.scalar.dma_start(qres_sb[:], io["qres"])
        wo_sb = latep.tile([128, DC, D], BF16)
        nc.scalar.dma_start(wo_sb[:], io["wo"])

        hsel = latep.tile([128, 1], F32)
        nc.sync.dma_start(hsel[:], io["hsel"])
        ctxAs = latep.tile([128, DC, KQ], F32)
        ctxBs = latep.tile([128, DC, KQ], F32)
        parts = latep.tile([128, 2, 2, DC, KQ], BF16)
        for r in range(2):
            for ab in range(2):
                nc.gpsimd.dma_start(parts[:, r, ab], cc_out[r, ab])
        # partner chunk = parts[1] + hsel*(parts[0]-parts[1]); hsel = my_half
        # (the partner of rank r is rank 1-r). ctx = own f32 partial + that.
        diff = latep.tile([128, 2, DC, KQ], F32)
        nc.vector.tensor_tensor(diff[:], parts[:, 0], parts[:, 1],
                                ALU.subtract)
        pick = latep.tile([128, 2, DC, KQ], F32)
        nc.vector.scalar_tensor_tensor(
            out=pick[:], in0=diff[:], scalar=hsel[:, 0:1],
            in1=parts[:, 1], op0=ALU.mult, op1=ALU.add)
        nc.vector.tensor_tensor(ctxAs[:], ctxA[:, :, 0:KQ], pick[:, 0],
                                ALU.add)
        nc.vector.tensor_tensor(ctxBs[:], ctxB[:, :, 0:KQ], pick[:, 1],
                                ALU.add)

        cxh = latep.tile([128, DC, KQ], BF16)
        dinv = latep.tile([1, 2, DC, KQ], F32)
        nc.vector.reciprocal(dinv[:, 0], ctxAs[64:65, :, :])
        nc.vector.reciprocal(dinv[:, 1], ctxBs[64:65, :, :])
        for h in range(H):
            hp, hh = h // 2, h % 2
            ctx_acc = ctxAs if hh == 0 else ctxBs
            rb = lscr.tile([128, KQ], F32, tag="recb")
            nc.gpsimd.partition_broadcast(rb[:], dinv[:, hh, hp, :])
            lo = hh * 64
            nc.vector.tensor_tensor(cxh[lo:lo + 64, hp, :],
                                    ctx_acc[0:64, hp, :],
                                    rb[0:64, :], ALU.mult)

        for s in range(2):
            for dh in range(2):
                po = psW.tile([128, 512], F32, tag="psW")
                for dc in range(DC):
                    nc.tensor.matmul(po[:], cxh[:, dc, s * 128:(s + 1) * 128],
                                     wo_sb[:, dc, dh * 512:(dh + 1) * 512],
                                     start=(dc == 0), stop=(dc == DC - 1))
                nc.vector.tensor_tensor(
                    out_attn[:, s, dh * 512:(dh + 1) * 512], po[:],
                    bo_bc[:, dh * 512:(dh + 1) * 512], ALU.add)
            nc.vector.tensor_tensor(out_attn[:, s, :], out_attn[:, s, :],
                                    qres_sb[:, s, :], ALU.add)

        # LN3 + transpose to z3T
        for s in range(2):
            stats = lscr.tile([128, 2, 6], F32, tag="bn3")
            nc.vector.bn_stats(stats[:, 0, :], out_attn[:, s, 0:512])
            nc.vector.bn_stats(stats[:, 1, :], out_attn[:, s, 512:1024])
            mv = lscr.tile([128, 2], F32, tag="mv3")
            nc.vector.bn_aggr(mv[:], stats[:])
            rstd = lscr.tile([128, 1], F32, tag="rstd3")
            nc.scalar.activation(out=rstd[:], in_=mv[:, 1:2], func=ACTF.Ln,
                                 bias=c_ln_eps[:], scale=1.0)
            nc.scalar.activation(out=rstd[:], in_=rstd[:], func=ACTF.Exp,
                                 bias=c_zero[:], scale=-0.5)
            nbias = lscr.tile([128, 1], F32, tag="nb3")
            nc.vector.tensor_tensor(nbias[:], mv[:, 0:1], rstd[:], ALU.mult)
            nc.vector.tensor_scalar_mul(nbias[:], nbias[:], -1.0)
            z3 = lscr.tile([128, 2, 512], BF16, tag="z3")
            for half in range(2):
                nc.scalar.activation(
                    out=z3[:, half, :],
                    in_=out_attn[:, s, half * 512:(half + 1) * 512],
                    func=ACTF.Identity, bias=nbias[:], scale=rstd[:])
            for dc in range(DC):
                pt = psT.tile([128, 128], BF16, tag="psT")
                nc.tensor.transpose(
                    pt[:], z3[:, dc // 4, (dc % 4) * 128:(dc % 4 + 1) * 128],
                    ident[:])
                nc.vector.tensor_copy(z3T[:, dc, s * 128:(s + 1) * 128], pt[:])

        if DEBUG:
            nc.sync.dma_start(io["dbg_ctx"], cxh[:])
            nc.sync.dma_start(io["dbg_attn"], out_attn[:])

    # ================= MLP super-phase =================
    with (
        tc.tile_pool(name="mw", bufs=6) as mw,
        tc.tile_pool(name="gt", bufs=2) as gtp,
        tc.tile_pool(name="mrow2", bufs=1) as mrow2,
        tc.tile_pool(name="psH", bufs=2, space="PSUM") as psH,
        tc.tile_pool(name="psO", bufs=1, space="PSUM") as psO,
    ):
        b2_bc = mrow2.tile([128, D], F32)
        nc.gpsimd.dma_start(
            out=b2_bc[:],
            in_=bass.AP(tensor=io["b2_row"].tensor, offset=io["b2_row"].offset,
                        ap=[[0, 128], io["b2_row"].ap[1]]),
        )
        pouts = {}
        for s in range(2):
            for dh in range(2):
                pouts[(s, dh)] = psO.tile([128, 512], F32, tag=f"po{s}{dh}",
                                          name=f"po{s}{dh}")
        nfc = FF // FC  # 8
        for fc in range(nfc):
            w1c = mw.tile([128, DC, FC], BF16, tag="w1")
            nc.sync.dma_start(w1c[:], io["w1"][fc])
            w2c = mw.tile([128, FC // 128, D], BF16, tag="w2")
            nc.sync.dma_start(w2c[:], io["w2"][fc])
            gt = gtp.tile([128, FC // 128, KQ], BF16, tag="gt")
            for fp in range(2):
                ph = psH.tile([128, 2, KQ], F32, tag="psH")
                for fi in range(2):
                    fs = fp * 2 + fi
                    for cc in range(DC):
                        nc.tensor.matmul(
                            ph[:, fi, :], w1c[:, cc, fs * 128:(fs + 1) * 128],
                            z3T[:, cc, :], start=(cc == 0), stop=(cc == DC - 1))
                fidx0 = fc * (FC // 128) + fp * 2
                for fi in range(2):
                    nc.scalar.activation(
                        out=gt[:, fp * 2 + fi, :], in_=ph[:, fi, :],
                        func=ACTF.Gelu,
                        bias=c1_sb[:, fidx0 + fi:fidx0 + fi + 1], scale=1.0)
            for s in range(2):
                for dh in range(2):
                    for fs in range(FC // 128):
                        nc.tensor.matmul(
                            pouts[(s, dh)][:], gt[:, fs, s * 128:(s + 1) * 128],
                            w2c[:, fs, dh * 512:(dh + 1) * 512],
                            start=(fc == 0 and fs == 0),
                            stop=(fc == nfc - 1 and fs == FC // 128 - 1))

        for s in range(2):
            for dh in range(2):
                sl = slice(dh * 512, (dh + 1) * 512)
                nc.vector.tensor_tensor(out_attn[:, s, sl], pouts[(s, dh)][:],
                                        out_attn[:, s, sl], ALU.add)
            nc.vector.tensor_tensor(out_attn[:, s, :], out_attn[:, s, :],
                                    b2_bc[:], ALU.add)
        nc.sync.dma_start(io["out"], out_attn[:])

    dram_cm.__exit__(None, None, None)


def build():
    nc = bacc.Bacc("TRN2", target_bir_lowering=False, debug=False,
                   num_devices=8)
    io = {}
    io["pf"] = [
        nc.dram_tensor(f"pf{j}", [128, DC, NB], BF16, kind="ExternalInput").ap()
        for j in range(NBLK)
    ]
    io["qt"] = nc.dram_tensor("qt", [128, DC, QF], BF16, kind="ExternalInput").ap()
    io["qres"] = nc.dram_tensor("qres", [128, 2, D], F32, kind="ExternalInput").ap()
    for w in ["wq", "wk", "wv", "wo"]:
        io[w] = nc.dram_tensor(w, [128, DC, D], BF16, kind="ExternalInput").ap()
    io["w1"] = [
        nc.dram_tensor(f"w1_{i}", [128, DC, FC], BF16, kind="ExternalInput").ap()
        for i in range(FF // FC)
    ]
    io["w2"] = [
        nc.dram_tensor(f"w2_{i}", [128, FC // 128, D], BF16,
                       kind="ExternalInput").ap()
        for i in range(FF // FC)
    ]
    io["ck"] = nc.dram_tensor("ck", [128, DC], F32, kind="ExternalInput").ap()
    io["cq"] = nc.dram_tensor("cq", [128, DC], F32, kind="ExternalInput").ap()
    io["wqk"] = nc.dram_tensor("wqk", [128, DC], F32, kind="ExternalInput").ap()
    io["c1"] = nc.dram_tensor("c1", [128, FF // 128], F32, kind="ExternalInput").ap()
    io["bo_row"] = nc.dram_tensor("bo_row", [1, D], F32, kind="ExternalInput").ap()
    io["hsel"] = nc.dram_tensor("hsel", [128, 1], F32, kind="ExternalInput").ap()
    io["b2_row"] = nc.dram_tensor("b2_row", [1, D], F32, kind="ExternalInput").ap()
    io["out"] = nc.dram_tensor("out", [128, 2, D], F32, kind="ExternalOutput").ap()
    if DEBUG:
        for name, shape, dt in [
            ("dbg_ctx", [128, DC, KQ], BF16),
            ("dbg_attn", [128, 2, D], F32),
        ]:
            io[name] = nc.dram_tensor(name, shape, dt, kind="ExternalOutput").ap()

    with tile.TileContext(nc) as tc:
        with tc.tile_pool(name="consts", bufs=1) as consts:
            _emit(nc, tc, io, consts)
    nc.compile()
    return nc


def prep_core_inputs(inputs, core):
    """Host-side fold + shard + relayout for one core."""
    b, half = core // 2, core % 2
    f32 = np.float32
    qt_full = np.asarray(inputs["query_tokens"], f32)
    pf_full = np.asarray(inputs["point_features"], f32)
    Wq = np.asarray(inputs["Wq"], f32)
    Wk = np.asarray(inputs["Wk"], f32)
    Wv = np.asarray(inputs["Wv"], f32)
    Wo = np.asarray(inputs["Wo"], f32)
    W1 = np.asarray(inputs["W1"], f32)
    W2 = np.asarray(inputs["W2"], f32)
    g_q, b_q = np.asarray(inputs["ln_q_g"], f32), np.asarray(inputs["ln_q_b"], f32)
    g_kv, b_kv = np.asarray(inputs["ln_kv_g"], f32), np.asarray(inputs["ln_kv_b"], f32)
    g_m, b_m = np.asarray(inputs["ln_mlp_g"], f32), np.asarray(inputs["ln_mlp_b"], f32)

    Wqp = g_q[:, None] * Wq
    c_q = b_q @ Wq + np.asarray(inputs["bq"], f32)
    Wkp = g_kv[:, None] * Wk
    c_k = b_kv @ Wk + np.asarray(inputs["bk"], f32)
    Wvp = g_kv[:, None] * Wv
    c_v = b_kv @ Wv + np.asarray(inputs["bv"], f32)
    W1p = g_m[:, None] * W1
    c_1 = b_m @ W1 + np.asarray(inputs["b1"], f32)
    wqk = (np.asarray(inputs["rms_q_w"], f32) * np.asarray(inputs["rms_k_w"], f32))
    bo_f = np.asarray(inputs["bo"], f32) + c_v @ Wo   # fold v-bias into bo

    q_res = qt_full[b, half * KQ:(half + 1) * KQ]          # own 256 rows
    pfT = np.ascontiguousarray(pf_full[b].T)               # [D, N]
    # all 512 queries, own half in local columns 0:KQ
    qord = np.roll(np.arange(2 * KQ), -half * KQ)
    qT = np.ascontiguousarray(qt_full[b].T[:, qord])       # [D, 512]

    def part_major(w, dt=NPBF):  # [D, X] -> [128, D//128, X]
        return np.ascontiguousarray(
            w.reshape(DC, 128, -1).transpose(1, 0, 2).astype(dt))

    m = {}
    # this core's kv half: blocks [half*NBLK, half*NBLK+NBLK)
    pf_dev = pfT.reshape(DC, 128, N // NB, NB).transpose(2, 1, 0, 3)
    for j in range(NBLK):
        m[f"pf{j}"] = np.ascontiguousarray(pf_dev[half * NBLK + j].astype(NPBF))
    m["qt"] = part_major(qT)
    m["qres"] = np.ascontiguousarray(q_res.reshape(2, 128, D).transpose(1, 0, 2))
    m["wq"] = part_major(Wqp)
    m["wk"] = part_major(Wkp)
    m["wv"] = part_major(Wvp)
    m["wo"] = part_major(Wo)
    w1_dev = part_major(W1p)                               # [128, 8, 4096]
    for i in range(FF // FC):
        m[f"w1_{i}"] = np.ascontiguousarray(w1_dev[:, :, i * FC:(i + 1) * FC])
    w2_dev = np.ascontiguousarray(
        W2.reshape(FF // 128, 128, D).transpose(1, 0, 2).astype(NPBF))
    for i in range(FF // FC):
        m[f"w2_{i}"] = np.ascontiguousarray(
            w2_dev[:, i * (FC // 128):(i + 1) * (FC // 128), :])
    m["ck"] = np.ascontiguousarray(c_k.reshape(DC, 128).T)
    m["cq"] = np.ascontiguousarray(c_q.reshape(DC, 128).T)
    m["wqk"] = np.ascontiguousarray(wqk.reshape(DC, 128).T)
    m["c1"] = np.ascontiguousarray(c_1.reshape(FF // 128, 128).T)
    m["bo_row"] = bo_f.reshape(1, D)
    m["hsel"] = np.full((128, 1), float(half), np.float32)
    m["b2_row"] = np.asarray(inputs["b2"], f32).reshape(1, D)
    return m


_NC_CACHE = None


def run_cores(inputs, **kw):
    global _NC_CACHE
    if _NC_CACHE is None:
        _NC_CACHE = build()
    in_maps = [prep_core_inputs(inputs, c) for c in range(8)]
    return run_bass_kernel_spmd(_NC_CACHE, in_maps, core_ids=list(range(8)), **kw)


def kernel(**inputs):
    res = run_cores(inputs)
    B, K = 4, 512
    out = np.zeros((B, K, D), np.float32)
    for c in range(8):
        b, half = c // 2, c % 2
        o = res.results[c]["out"]                          # [128, 2, 1024]
        out[b, half * KQ:(half + 1) * KQ] = o.transpose(1, 0, 2).reshape(KQ, D)
    return out
